# revision 6
# baseline (speedup 1.0000x reference)
"""Trainium2 Bass kernel for nn_CrossTransLayer (Nystrom-style landmark attention).

Sharding: 8 cores = 4 batches x 2 head-groups (4 heads each).
Each core computes its batch's attention for its 4 heads plus the partial
output projection; the host sums the two head-group partials per batch.

All device math is done in "feature-on-partition" (transposed) layouts so the
contraction dim of every matmul is already on partitions:
  xT [DIM, N], zT [DIM, NZ] fed from host in bf16 (host transposes once).
Softmax row-sums are obtained with appended all-ones matmul rows/columns, so
no cross-partition reductions are ever needed, and per-token normalization of
the transposed attention output uses a gpsimd partition-broadcast of the
reciprocal row.

Matmul operands are bf16 (1 cyc/row on PE, half the LDWEIGHTS traffic)
everywhere except the Newton-Schulz pinv iteration, which stays float32r to
track the reference's f32 iteration path. Reciprocals use the table-free
RECIPROCAL_APPROX_FAST custom DVE op (~51 ULP).

The reference's Newton-Schulz pinv scales z0 by a GLOBAL (over all b,h) max of
attn2 row/col sums; the iteration does not converge in 6 steps, so that scale
must match exactly. The host replicates the (tiny) landmark pipeline in numpy
to produce that one scalar, passed in as c0.
"""

import os

import ml_dtypes
import numpy as np

os.environ.setdefault("MYCRO_LOCAL_CACHE", "1")

import concourse.bass as bass
import concourse.mybir as mybir
import concourse.tile as tile
from concourse import bacc
from concourse.bass_utils import run_bass_kernel_spmd
from concourse.masks import make_identity

F32 = mybir.dt.float32
F32R = mybir.dt.float32r
BF16 = mybir.dt.bfloat16

B, N, NZ, DIM = 4, 8192, 4096, 512
H, DH, M = 8, 64, 256
HPC = 4               # heads per core
GD = HPC * DH         # 256 head-dims per core
L, LZ = N // M, NZ // M  # 32, 16
SCALE = DH ** -0.5
N_CORES = 8
P = 128

AF = mybir.ActivationFunctionType
ALU = mybir.AluOpType

LAST_RESULTS = None  # BassKernelResults of the most recent run (for test harness)
LAST_IN_MAPS = None


def _emit(nc):
    xT = nc.dram_tensor("xT", [DIM, N], BF16, kind="ExternalInput")
    zT = nc.dram_tensor("zT", [DIM, NZ], BF16, kind="ExternalInput")
    w3 = nc.dram_tensor("w3", [DIM, 3 * GD], BF16, kind="ExternalInput")
    wo = nc.dram_tensor("wo", [GD, DIM], BF16, kind="ExternalInput")
    c0d = nc.dram_tensor("c0", [1, 1], F32, kind="ExternalInput")
    bb = nc.dram_tensor("bb", [1, DIM], F32, kind="ExternalInput")
    y = nc.dram_tensor("y", [N, DIM], BF16, kind="ExternalOutput")

    KD = DIM // P  # 4 K-tiles over DIM

    with tile.TileContext(nc) as tc:
        const = tc.alloc_tile_pool(name="const", bufs=1)
        persist = tc.alloc_tile_pool(name="persist", bufs=1)

        # ---- constants ----
        w3_sb = const.tile([P, KD, 3 * GD], BF16, tag="w3")
        wo_sb = const.tile([P, 2, DIM], BF16, tag="wo")
        for k in range(KD):
            nc.scalar.dma_start(out=w3_sb[:, k, :], in_=w3[k * P:(k + 1) * P, :])
        for k in range(2):
            nc.scalar.dma_start(out=wo_sb[:, k, :], in_=wo[k * P:(k + 1) * P, :])
        wq_sb = w3_sb[:, :, 0:GD]
        wk_sb = w3_sb[:, :, GD:2 * GD]
        wv_sb = w3_sb[:, :, 2 * GD:3 * GD]

        c0row = const.tile([1, 1], F32, tag="c0row")
        nc.scalar.dma_start(out=c0row[:], in_=c0d[:])
        c0col = const.tile([P, 1], F32, tag="c0col")
        nc.gpsimd.partition_broadcast(c0col[:], c0row[:])
        bbrow = const.tile([1, DIM], F32, tag="bbrow")
        nc.scalar.dma_start(out=bbrow[:], in_=bb[:])
        bbcast = const.tile([P, DIM], F32, tag="bbcast")
        nc.gpsimd.partition_broadcast(bbcast[:], bbrow[:])

        id128 = const.tile([P, P], F32, tag="id128")
        make_identity(nc, id128[:])
        id128r = const.tile([P, P], F32R, tag="id128r")
        nc.vector.tensor_copy(id128r[:], id128[:])
        ones_st = const.tile([P, HPC, 2], F32, tag="ones_st")
        nc.vector.memset(ones_st[:, :, 0:1], 1.0)
        nc.vector.memset(ones_st[:, :, 1:2], 0.0)
        onez_b = const.tile([P, HPC, 2], BF16, tag="onez_b")
        nc.vector.tensor_copy(onez_b[:], ones_st[:])
        # packed [256,256] scaled identities for the pinv polynomial:
        # packed[:, mb*256:(mb+1)*256] holds matrix rows mb*128..
        ids = {}
        for nm, val in (("i7", 7.0), ("i15", 15.0), ("i325", 3.25)):
            t = const.tile([P, 2 * M], F32, tag=nm)
            nc.vector.memset(t[:], 0.0)
            for mb in range(2):
                off = mb * M + mb * P
                nc.scalar.mul(out=t[:, off:off + P], in_=id128[:], mul=val)
            ids[nm] = t

        # ---- persistent tensors ----
        qt_sb = [persist.tile([P, N], BF16, tag=f"qt{p}", name=f"qt{p}") for p in range(2)]
        # f32 landmark tiles (for the sim2/pinv path) + bf16 copies (sim1/sim3)
        q_lT = [persist.tile([P, M], F32R, tag=f"qlt{p}", name=f"qlt{p}") for p in range(2)]
        k_lT = [persist.tile([P, M], F32R, tag=f"klt{p}", name=f"klt{p}") for p in range(2)]
        q_lTb = [persist.tile([P, M], BF16, tag=f"qltb{p}", name=f"qltb{p}") for p in range(2)]
        k_lTb = [persist.tile([P, M], BF16, tag=f"kltb{p}", name=f"kltb{p}") for p in range(2)]
        # A3V per head: [256,64] stored as [128, h, kblock, 64]
        a3v_sb = persist.tile([P, HPC, 2, DH], BF16, tag="a3v")
        # W2e per head: [W2' | ones] as lhsT blocks [128, h, kblock, 65]
        w2e_sb = persist.tile([P, HPC, 2, DH + 2], BF16, tag="w2e")
        # pinv per-head state (packed [256,256] -> [128, 512])
        aT_sb = [persist.tile([P, 2 * M], F32R, tag=f"aT{h}", name=f"aT{h}") for h in range(HPC)]
        zt_sb = [persist.tile([P, 2 * M], F32R, tag=f"zt{h}", name=f"zt{h}") for h in range(HPC)]
        ztT_sb = [persist.tile([P, 2 * M], F32R, tag=f"ztT{h}", name=f"ztT{h}") for h in range(HPC)]

        # ================= phase A1: qT + landmark means =================
        with tc.tile_pool(name="a1", bufs=2) as a1, \
             tc.tile_pool(name="a1ps", bufs=3, space="PSUM") as a1ps:
            CH = 512
            GCH = 2048  # DMA group width
            for g in range(N // GCH):
                xc = [a1.tile([P, GCH], BF16, tag=f"xc{k}", name=f"xc{k}") for k in range(KD)]
                for k in range(KD):
                    eng = nc.sync if k % 2 == 0 else nc.scalar
                    eng.dma_start(
                        out=xc[k][:],
                        in_=xT[k * P:(k + 1) * P, g * GCH:(g + 1) * GCH])
                for sub in range(GCH // CH):
                    c = g * (GCH // CH) + sub
                    for p in range(2):
                        qps = a1ps.tile([P, CH], F32, tag="qps")
                        for k in range(KD):
                            nc.tensor.matmul(
                                qps[:], wq_sb[:, k, p * P:(p + 1) * P],
                                xc[k][:, sub * CH:(sub + 1) * CH],
                                start=(k == 0), stop=(k == KD - 1))
                        nc.scalar.copy(out=qt_sb[p][:, c * CH:(c + 1) * CH],
                                       in_=qps[:])
            for p in range(2):
                qsum = a1.tile([P, M], F32, tag="qsum")
                nc.vector.tensor_reduce(
                    out=qsum[:],
                    in_=qt_sb[p][:].rearrange("p (m l) -> p m l", l=L),
                    axis=mybir.AxisListType.X, op=ALU.add)
                nc.vector.tensor_scalar_mul(q_lT[p][:], qsum[:], 1.0 / L)
                nc.vector.tensor_copy(q_lTb[p][:], q_lT[p][:])

        # ====== phase A2: kT/v stream, k_land, sim3T/E3T -> P3 -> A3V ======
        with tc.tile_pool(name="a2", bufs=2) as a2, \
             tc.tile_pool(name="a2ps", bufs=2, space="PSUM") as a2ps, \
             tc.tile_pool(name="p3ps", bufs=2, space="PSUM") as p3ps:
            W = DH + 2  # 64 data + ones col + pad
            # SBUF accumulators for P3 = E3 @ [v|1]  (one per head)
            p3a = [a2.tile([P, 2 * W], F32, tag=f"p3a{h}", name=f"p3a{h}",
                           bufs=1) for h in range(HPC)]
            for h in range(HPC):
                nc.vector.memset(p3a[h][:], 0.0)
            CH = 512
            NT = CH // P  # token-tiles per chunk
            GCH = 2048
            zcb = {}
            for c in range(NZ // CH):
                g, sub = divmod(c, GCH // CH)
                if sub == 0:
                    zcb = {k: a2.tile([P, GCH], BF16, tag=f"zc{k}",
                                      name=f"zc{k}", bufs=1) for k in range(KD)}
                    for k in range(KD):
                        eng = nc.sync if k % 2 == 0 else nc.scalar
                        eng.dma_start(
                            out=zcb[k][:],
                            in_=zT[k * P:(k + 1) * P, g * GCH:(g + 1) * GCH])
                zc = [zcb[k][:, sub * CH:(sub + 1) * CH] for k in range(KD)]
                # kT chunk for both packs
                ktc = []
                for p in range(2):
                    kps = a2ps.tile([P, CH], F32, tag="mmps")
                    for k in range(KD):
                        nc.tensor.matmul(
                            kps[:], wk_sb[:, k, p * P:(p + 1) * P],
                            zc[k], start=(k == 0), stop=(k == KD - 1))
                    ksb = a2.tile([P, CH], BF16, tag=f"ktc{p}")
                    nc.scalar.copy(out=ksb[:], in_=kps[:])
                    ktc.append(ksb)
                    # partial k_land: this chunk covers landmarks c*32..c*32+31
                    ksum = a2.tile([P, CH // LZ], F32, tag=f"ksum{p}")
                    nc.vector.tensor_reduce(
                        out=ksum[:],
                        in_=kps[:].rearrange("p (m l) -> p m l", l=LZ),
                        axis=mybir.AxisListType.X, op=ALU.add)
                    nc.vector.tensor_scalar_mul(
                        k_lT[p][:, c * (CH // LZ):(c + 1) * (CH // LZ)],
                        ksum[:], 1.0 / LZ)
                # v chunk: [tok, 4 heads * 64] -> v_sb [128, h, 66] per tile
                vtiles = []
                for t in range(NT):
                    vps = a2ps.tile([P, GD], F32, tag="mmps")
                    for k in range(KD):
                        nc.tensor.matmul(
                            vps[:], zc[k][:, t * P:(t + 1) * P],
                            wv_sb[:, k, :], start=(k == 0),
                            stop=(k == KD - 1))
                    vsb = a2.tile([P, HPC, W], BF16, tag=f"vsb{t}")
                    nc.vector.tensor_copy(
                        vsb[:, :, 0:DH],
                        vps[:].rearrange("p (h d) -> p h d", d=DH))
                    nc.vector.tensor_copy(vsb[:, :, DH:W], onez_b[:])
                    vtiles.append(vsb)
                # sim3T / E3T -> chunk-partial P3 (sequential mb groups per
                # head keep one open psum accumulation group per bank)
                for h in range(HPC):
                    p, off = h // 2, (h % 2) * DH
                    e3s = []
                    for t in range(NT):
                        s3 = a2ps.tile([P, M], F32, tag="s3ps")
                        nc.tensor.matmul(
                            s3[:],
                            ktc[p][off:off + DH, t * P:(t + 1) * P],
                            q_lTb[p][off:off + DH, :],
                            start=True, stop=True)
                        e3 = a2.tile([P, M], BF16, tag="e3", bufs=2 * NT)
                        nc.scalar.activation(e3[:], s3[:], AF.Exp)
                        e3s.append(e3)
                    p3c = p3ps.tile([P, 2 * W], F32, tag="p3c")
                    for mb in range(2):
                        for t in range(NT):
                            nc.tensor.matmul(
                                p3c[:, mb * W:(mb + 1) * W],
                                e3s[t][:, mb * P:(mb + 1) * P],
                                vtiles[t][:, h, :],
                                start=(t == 0), stop=(t == NT - 1))
                    nc.vector.tensor_add(p3a[h][:], p3a[h][:], p3c[:])
            for p in range(2):
                nc.vector.tensor_copy(k_lTb[p][:], k_lT[p][:])
            # P3 -> A3V  (attn3 @ v with softmax denom from the ones column)
            for h in range(HPC):
                p3v = p3a[h][:].rearrange("p (m w) -> p m w", w=W)
                r3 = a2.tile([P, 2], F32, tag="r3")
                for mb in range(2):
                    nc.vector.reciprocal(
                        r3[:, mb:mb + 1], p3v[:, mb, DH:DH + 1])
                    nc.vector.tensor_scalar_mul(
                        a3v_sb[:, h, mb, :], p3v[:, mb, 0:DH], r3[:, mb:mb + 1])

        # ============ phase B: sim2, attn2(T), pinv, W2e per head ============
        with tc.tile_pool(name="pb", bufs=2) as pb, \
             tc.tile_pool(name="bmis", bufs=1, space="PSUM") as bmis, \
             tc.tile_pool(name="bpin", bufs=1, space="PSUM") as bpin:
            for h in range(HPC):
                p, off = h // 2, (h % 2) * DH
                qh = q_lT[p][off:off + DH, :]   # [64, 256]
                kh = k_lT[p][off:off + DH, :]   # [64, 256]

                sim2 = bmis.tile([P, 2 * M], F32, tag="bmm")
                for mb in range(2):
                    nc.tensor.matmul(
                        sim2[:, mb * M:(mb + 1) * M],
                        qh[:, mb * P:(mb + 1) * P], kh,
                        start=True, stop=True)
                e2 = pb.tile([P, 2 * M], F32, tag="e2")
                s2 = pb.tile([P, 2], F32, tag="s2")
                for mb in range(2):
                    nc.scalar.activation(
                        e2[:, mb * M:(mb + 1) * M], sim2[:, mb * M:(mb + 1) * M],
                        AF.Exp, accum_out=s2[:, mb:mb + 1])
                r2 = pb.tile([P, 2], F32, tag="r2")
                nc.vector.reciprocal(r2[:], s2[:])

                sim2t = bmis.tile([P, 2 * M], F32, tag="bmm")
                for mb in range(2):
                    nc.tensor.matmul(
                        sim2t[:, mb * M:(mb + 1) * M],
                        kh[:, mb * P:(mb + 1) * P], qh,
                        start=True, stop=True)
                e2t = pb.tile([P, 2 * M], F32, tag="e2t")
                nc.scalar.activation(e2t[:], sim2t[:], AF.Exp)

                # row-vector of 1/s2 via PE transpose, then partition bcast
                trp = bmis.tile([1, M], F32, tag="trp")
                for mb in range(2):
                    nc.tensor.transpose(
                        trp[0:1, mb * P:(mb + 1) * P], r2[:, mb:mb + 1],
                        id128[:])
                r2row = pb.tile([1, M], F32, tag="r2row")
                nc.vector.tensor_copy(r2row[:], trp[:])
                r2b = pb.tile([P, M], F32, tag="r2b")
                nc.gpsimd.partition_broadcast(r2b[:], r2row[:])

                # aT = attn2^T ; zt0 = aT*c0 ; ztT0 = attn2*c0
                for mb in range(2):
                    nc.vector.tensor_mul(
                        aT_sb[h][:, mb * M:(mb + 1) * M],
                        e2t[:, mb * M:(mb + 1) * M], r2b[:])
                    nc.vector.tensor_scalar(
                        ztT_sb[h][:, mb * M:(mb + 1) * M],
                        e2[:, mb * M:(mb + 1) * M],
                        r2[:, mb:mb + 1], c0col[:, 0:1],
                        ALU.mult, ALU.mult)
                nc.vector.tensor_scalar_mul(
                    zt_sb[h][:], aT_sb[h][:], c0col[:, 0:1])

            def mm256(out_ps, lhsT_pk, rhs_pk):
                """[256,256] @ [256,256] in packed [128,512] layout."""
                for mb in range(2):
                    for k in range(2):
                        nc.tensor.matmul(
                            out_ps[:, mb * M:(mb + 1) * M],
                            lhsT_pk[:, k * M + mb * P:k * M + (mb + 1) * P],
                            rhs_pk[:, k * M:(k + 1) * M],
                            start=(k == 0), stop=(k == 1))

            ITERS = 6
            for it in range(ITERS):
                for h in range(HPC):
                    az = bpin.tile([P, 2 * M], F32, tag="az")
                    mm256(az, aT_sb[h][:], zt_sb[h][:])
                    azt_ps = bpin.tile([P, 2 * M], F32, tag="azt")
                    mm256(azt_ps, zt_sb[h][:], aT_sb[h][:])
                    azt = pb.tile([P, 2 * M], F32R, tag="azt_sb")
                    nc.scalar.copy(out=azt[:], in_=azt_ps[:])
                    t1 = pb.tile([P, 2 * M], F32R, tag="t1")
                    nc.vector.tensor_sub(t1[:], ids["i7"][:], az[:])
                    t2 = bpin.tile([P, 2 * M], F32, tag="t2")
                    mm256(t2, azt[:], t1[:])
                    t3 = pb.tile([P, 2 * M], F32R, tag="t3")
                    nc.vector.tensor_sub(t3[:], ids["i15"][:], t2[:])
                    t4 = bpin.tile([P, 2 * M], F32, tag="t4")
                    mm256(t4, azt[:], t3[:])
                    t5 = pb.tile([P, 2 * M], F32R, tag="t5")
                    nc.vector.scalar_tensor_tensor(
                        t5[:], t4[:], -0.25, ids["i325"][:],
                        ALU.mult, ALU.add)
                    znew = bpin.tile([P, 2 * M], F32, tag="znew")
                    mm256(znew, ztT_sb[h][:], t5[:])
                    nc.scalar.copy(out=zt_sb[h][:], in_=znew[:])
                    # ztT via PE transpose of the 4 [128,128] blocks
                    ztt_ps = bpin.tile([P, 2 * M], F32R, tag="zttp")
                    for mb in range(2):
                        for k in range(2):
                            nc.tensor.transpose(
                                ztt_ps[:, mb * M + k * P:mb * M + (k + 1) * P],
                                zt_sb[h][:, k * M + mb * P:k * M + (mb + 1) * P],
                                id128r[:])
                    nc.scalar.copy(out=ztT_sb[h][:], in_=ztt_ps[:])

            # W2' = zt @ A3V ; W2e = [W2' | ones] as lhsT [256 -> k, 65]
            for h in range(HPC):
                a3vr = pb.tile([P, 2, DH], F32R, tag="a3vr")
                nc.vector.tensor_copy(a3vr[:], a3v_sb[:, h, :, :])
                w2p = bmis.tile([P, 2 * DH], F32, tag="bmm")
                for mb in range(2):
                    for k in range(2):
                        nc.tensor.matmul(
                            w2p[:, mb * DH:(mb + 1) * DH],
                            ztT_sb[h][:, k * M + mb * P:k * M + (mb + 1) * P],
                            a3vr[:, k, :],
                            start=(k == 0), stop=(k == 1))
                nc.vector.tensor_copy(
                    w2e_sb[:, h, :, 0:DH],
                    w2p[:].rearrange("p (m d) -> p m d", d=DH))
                nc.vector.tensor_copy(w2e_sb[:, h, :, DH:DH + 2], onez_b[:, 0:2, :])

        # ========= phase C: attn1(T) + out projection + residual =========
        with tc.tile_pool(name="pc", bufs=2) as pc, \
             tc.tile_pool(name="cps", bufs=2, space="PSUM") as cps, \
             tc.tile_pool(name="s1ps", bufs=4, space="PSUM") as s1ps:
            CH = 512
            for c in range(N // CH):
                ot = [pc.tile([P, CH], BF16, tag=f"ot{k}", name=f"ot{k}") for k in range(2)]
                for h in range(HPC):
                    p, off = h // 2, (h % 2) * DH
                    e1 = []
                    for mb in range(2):
                        s1 = s1ps.tile([P, CH], F32, tag="s1")
                        nc.tensor.matmul(
                            s1[:],
                            k_lTb[p][off:off + DH, mb * P:(mb + 1) * P],
                            qt_sb[p][off:off + DH, c * CH:(c + 1) * CH],
                            start=True, stop=True)
                        e = pc.tile([P, CH], BF16, tag="e1", bufs=8)
                        nc.scalar.activation(e[:], s1[:], AF.Exp)
                        e1.append(e)
                    otp = cps.tile([P, CH], F32, tag="otp")
                    for k in range(2):
                        nc.tensor.matmul(
                            otp[0:DH + 2, :], w2e_sb[:, h, k, :],
                            e1[k][:], start=(k == 0), stop=(k == 1))
                    r1 = pc.tile([1, CH], F32, tag="r1", bufs=4)
                    nc.vector.reciprocal(r1[:], otp[DH:DH + 1, :])
                    s1b = pc.tile([DH, CH], F32, tag="s1b", bufs=4)
                    nc.gpsimd.partition_broadcast(s1b[:], r1[:])
                    nc.vector.tensor_mul(
                        ot[p][off:off + DH, :], otp[0:DH, :], s1b[:])
                # projection: y[tok, :] = outT.T @ wo + b_out (residual on host)
                ysb = pc.tile([P, CH // P, DIM], BF16, tag="ysb")
                for s in range(CH // P):
                    yps = cps.tile([P, DIM], F32, tag="yps")
                    for k in range(2):
                        nc.tensor.matmul(
                            yps[:], ot[k][:, s * P:(s + 1) * P],
                            wo_sb[:, k, :], start=(k == 0), stop=(k == 1))
                    nc.vector.tensor_add(ysb[:, s, :], yps[:], bbcast[:])
                nc.sync.dma_start(
                    out=y[c * CH:(c + 1) * CH, :].rearrange(
                        "(s p) d -> p s d", p=P),
                    in_=ysb[:])

        persist.release()
        const.release()
    return nc


_BUILT = None


def _build():
    global _BUILT
    if _BUILT is None:
        nc = bacc.Bacc("TRN2", target_bir_lowering=False, debug=False)
        _emit(nc)
        nc.finalize()
        _BUILT = nc
    return _BUILT


def _host_c0(x, z, W_q, W_kv):
    """Replicate the reference's global pinv z0 scale (one scalar)."""
    x_land = x.reshape(B, M, L, DIM).mean(2)
    z_land = z.reshape(B, M, LZ, DIM).mean(2)
    q_land = (x_land @ W_q) * SCALE          # [B, M, H*DH]
    k_land = z_land @ W_kv[:, :H * DH]
    q_land = q_land.reshape(B, M, H, DH).transpose(0, 2, 1, 3)
    k_land = k_land.reshape(B, M, H, DH).transpose(0, 2, 1, 3)
    sim2 = q_land @ k_land.transpose(0, 1, 3, 2)   # [B, H, M, M]
    e = np.exp(sim2 - sim2.max(-1, keepdims=True))
    attn2 = e / e.sum(-1, keepdims=True)
    col = np.abs(attn2).sum(-1)
    row = np.abs(attn2).sum(-2)
    return np.float32(1.0) / (col.max() * row.max())


def kernel(x, z, W_q, W_kv, W_out, b_out):
    global LAST_RESULTS, LAST_IN_MAPS
    x = np.asarray(x, dtype=np.float32)
    z = np.asarray(z, dtype=np.float32)
    W_q = np.asarray(W_q, dtype=np.float32)
    W_kv = np.asarray(W_kv, dtype=np.float32)
    W_out = np.asarray(W_out, dtype=np.float32)
    b_out = np.asarray(b_out, dtype=np.float32)

    c0 = np.array([[_host_c0(x, z, W_q, W_kv)]], dtype=np.float32)
    zeros_row = np.zeros((1, DIM), dtype=np.float32)
    bb_row = b_out.reshape(1, DIM).astype(np.float32)

    bf = ml_dtypes.bfloat16
    in_maps = []
    for i in range(N_CORES):
        bi, g = i // 2, i % 2
        cs = slice(g * GD, (g + 1) * GD)
        w3cat = np.concatenate(
            [W_q[:, cs] * SCALE, W_kv[:, cs], W_kv[:, H * DH:][:, cs]], axis=1)
        in_maps.append({
            "xT": np.ascontiguousarray(x[bi].T).astype(bf),
            "zT": np.ascontiguousarray(z[bi].T).astype(bf),
            "w3": np.ascontiguousarray(w3cat).astype(bf),
            "wo": np.ascontiguousarray(W_out[cs, :]).astype(bf),
            "c0": c0,
            "bb": zeros_row if g == 0 else bb_row,
        })

    nc = _build()
    LAST_IN_MAPS = in_maps
    LAST_RESULTS = run_bass_kernel_spmd(
        nc, in_maps, list(range(N_CORES)),
        tmpdir=os.environ.get("KERNEL_TRACE_DIR") or None)
    rs = LAST_RESULTS.results

    out = np.empty((B, N, DIM), dtype=np.float32)
    for bi in range(B):
        np.add(rs[2 * bi]["y"].astype(np.float32),
               rs[2 * bi + 1]["y"].astype(np.float32), out=out[bi])
        out[bi] += x[bi]
    return out


# revision 10
# speedup vs baseline: 1.0072x; 1.0072x over previous
"""Trainium2 Bass kernel for nn_CrossTransLayer (Nystrom-style landmark attention).

Sharding: 8 cores = 4 batches x 2 head-groups (4 heads each).
Each core computes its batch's attention for its 4 heads plus the partial
output projection; the host sums the two head-group partials per batch.

All device math is done in "feature-on-partition" (transposed) layouts so the
contraction dim of every matmul is already on partitions:
  xT [DIM, N], zT [DIM, NZ] fed from host in bf16 (host transposes once).
Softmax row-sums are obtained with appended all-ones matmul rows/columns, so
no cross-partition reductions are ever needed, and per-token normalization of
the transposed attention output uses a gpsimd partition-broadcast of the
reciprocal row.

Matmul operands are bf16 (1 cyc/row on PE, half the LDWEIGHTS traffic)
everywhere except the Newton-Schulz pinv iteration, which stays float32r to
track the reference's f32 iteration path. Reciprocals use the table-free
RECIPROCAL_APPROX_FAST custom DVE op (~51 ULP).

The reference's Newton-Schulz pinv scales z0 by a GLOBAL (over all b,h) max of
attn2 row/col sums; the iteration does not converge in 6 steps, so that scale
must match exactly. The host replicates the (tiny) landmark pipeline in numpy
to produce that one scalar, passed in as c0.
"""

import os

import ml_dtypes
import numpy as np

os.environ.setdefault("MYCRO_LOCAL_CACHE", "1")

import concourse.bass as bass
import concourse.mybir as mybir
import concourse.tile as tile
from concourse import bacc
from concourse.bass_utils import run_bass_kernel_spmd
from concourse.masks import make_identity

F32 = mybir.dt.float32
F32R = mybir.dt.float32r
BF16 = mybir.dt.bfloat16

B, N, NZ, DIM = 4, 8192, 4096, 512
H, DH, M = 8, 64, 256
HPC = 4               # heads per core
GD = HPC * DH         # 256 head-dims per core
L, LZ = N // M, NZ // M  # 32, 16
SCALE = DH ** -0.5
N_CORES = 8
P = 128

AF = mybir.ActivationFunctionType
ALU = mybir.AluOpType

LAST_RESULTS = None  # BassKernelResults of the most recent run (for test harness)
LAST_IN_MAPS = None


def _emit(nc):
    xT = nc.dram_tensor("xT", [DIM, N], BF16, kind="ExternalInput")
    zT = nc.dram_tensor("zT", [DIM, NZ], BF16, kind="ExternalInput")
    w3 = nc.dram_tensor("w3", [DIM, 3 * GD], BF16, kind="ExternalInput")
    wo = nc.dram_tensor("wo", [GD, DIM], BF16, kind="ExternalInput")
    c0d = nc.dram_tensor("c0", [1, 1], F32, kind="ExternalInput")
    bb = nc.dram_tensor("bb", [1, DIM], F32, kind="ExternalInput")
    y = nc.dram_tensor("y", [N, DIM], BF16, kind="ExternalOutput")

    KD = DIM // P  # 4 K-tiles over DIM

    with tile.TileContext(nc) as tc:
        const = tc.alloc_tile_pool(name="const", bufs=1)
        persist = tc.alloc_tile_pool(name="persist", bufs=1)

        # ---- constants ----
        w3_sb = const.tile([P, KD, 3 * GD], BF16, tag="w3")
        wo_sb = const.tile([P, 2, DIM], BF16, tag="wo")
        for k in range(KD):
            nc.scalar.dma_start(out=w3_sb[:, k, :], in_=w3[k * P:(k + 1) * P, :])
        for k in range(2):
            nc.scalar.dma_start(out=wo_sb[:, k, :], in_=wo[k * P:(k + 1) * P, :])
        wq_sb = w3_sb[:, :, 0:GD]
        wk_sb = w3_sb[:, :, GD:2 * GD]
        wv_sb = w3_sb[:, :, 2 * GD:3 * GD]

        c0row = const.tile([1, 1], F32, tag="c0row")
        nc.scalar.dma_start(out=c0row[:], in_=c0d[:])
        c0col = const.tile([P, 1], F32, tag="c0col")
        nc.gpsimd.partition_broadcast(c0col[:], c0row[:])
        bbrow = const.tile([1, DIM], F32, tag="bbrow")
        nc.scalar.dma_start(out=bbrow[:], in_=bb[:])
        bbcast = const.tile([P, DIM], F32, tag="bbcast")
        nc.gpsimd.partition_broadcast(bbcast[:], bbrow[:])

        id128 = const.tile([P, P], F32, tag="id128")
        make_identity(nc, id128[:])
        id128b = const.tile([P, P], BF16, tag="id128b")
        nc.vector.tensor_copy(id128b[:], id128[:])
        ones_st = const.tile([P, HPC, 2], F32, tag="ones_st")
        nc.vector.memset(ones_st[:, :, 0:1], 1.0)
        nc.vector.memset(ones_st[:, :, 1:2], 0.0)
        onez_b = const.tile([P, HPC, 2], BF16, tag="onez_b")
        nc.vector.tensor_copy(onez_b[:], ones_st[:])
        # packed [256,256] scaled identities for the pinv polynomial:
        # packed[:, mb*256:(mb+1)*256] holds matrix rows mb*128..
        ids = {}
        for nm, val in (("i7", 7.0), ("i15", 15.0), ("i325", 3.25)):
            t = const.tile([P, 2 * M], F32, tag=nm)
            nc.vector.memset(t[:], 0.0)
            for mb in range(2):
                off = mb * M + mb * P
                nc.scalar.mul(out=t[:, off:off + P], in_=id128[:], mul=val)
            ids[nm] = t

        # ---- persistent tensors ----
        qt_sb = [persist.tile([P, N], BF16, tag=f"qt{p}", name=f"qt{p}") for p in range(2)]
        # f32 landmark tiles (for the sim2/pinv path) + bf16 copies (sim1/sim3)
        q_lT = [persist.tile([P, M], F32R, tag=f"qlt{p}", name=f"qlt{p}") for p in range(2)]
        k_lT = [persist.tile([P, M], F32R, tag=f"klt{p}", name=f"klt{p}") for p in range(2)]
        q_lTb = [persist.tile([P, M], BF16, tag=f"qltb{p}", name=f"qltb{p}") for p in range(2)]
        k_lTb = [persist.tile([P, M], BF16, tag=f"kltb{p}", name=f"kltb{p}") for p in range(2)]
        # A3V per head: [256,64] stored as [128, h, kblock, 64]
        a3v_sb = persist.tile([P, HPC, 2, DH], BF16, tag="a3v")
        # W2e per head: [W2' | ones] as lhsT blocks [128, h, kblock, 65]
        w2e_sb = persist.tile([P, HPC, 2, DH + 2], BF16, tag="w2e")
        # pinv per-head state (packed [256,256] -> [128, 512]), bf16 matmul
        # operands; psum + polynomial combines stay f32
        aT_sb = [persist.tile([P, 2 * M], BF16, tag=f"aT{h}", name=f"aT{h}") for h in range(HPC)]
        zt_sb = [persist.tile([P, 2 * M], BF16, tag=f"zt{h}", name=f"zt{h}") for h in range(HPC)]
        ztT_sb = [persist.tile([P, 2 * M], BF16, tag=f"ztT{h}", name=f"ztT{h}") for h in range(HPC)]

        # ================= phase A1: qT + landmark means =================
        with tc.tile_pool(name="a1", bufs=2) as a1, \
             tc.tile_pool(name="a1ps", bufs=3, space="PSUM") as a1ps:
            CH = 512
            GCH = 2048  # DMA group width
            for g in range(N // GCH):
                xc = [a1.tile([P, GCH], BF16, tag=f"xc{k}", name=f"xc{k}") for k in range(KD)]
                for k in range(KD):
                    eng = nc.sync if k % 2 == 0 else nc.scalar
                    eng.dma_start(
                        out=xc[k][:],
                        in_=xT[k * P:(k + 1) * P, g * GCH:(g + 1) * GCH])
                for sub in range(GCH // CH):
                    c = g * (GCH // CH) + sub
                    for p in range(2):
                        qps = a1ps.tile([P, CH], F32, tag="qps")
                        for k in range(KD):
                            nc.tensor.matmul(
                                qps[:], wq_sb[:, k, p * P:(p + 1) * P],
                                xc[k][:, sub * CH:(sub + 1) * CH],
                                start=(k == 0), stop=(k == KD - 1))
                        nc.scalar.copy(out=qt_sb[p][:, c * CH:(c + 1) * CH],
                                       in_=qps[:])
            for p in range(2):
                qsum = a1.tile([P, M], F32, tag="qsum")
                nc.vector.tensor_reduce(
                    out=qsum[:],
                    in_=qt_sb[p][:].rearrange("p (m l) -> p m l", l=L),
                    axis=mybir.AxisListType.X, op=ALU.add)
                nc.vector.tensor_scalar_mul(q_lT[p][:], qsum[:], 1.0 / L)
                nc.vector.tensor_copy(q_lTb[p][:], q_lT[p][:])

        # ====== phase A2: kT/v stream, k_land, sim3T/E3T -> P3 -> A3V ======
        with tc.tile_pool(name="a2", bufs=2) as a2, \
             tc.tile_pool(name="a2ps", bufs=2, space="PSUM") as a2ps, \
             tc.tile_pool(name="p3ps", bufs=2, space="PSUM") as p3ps:
            W = DH + 2  # 64 data + ones col + pad
            # SBUF accumulators for P3 = E3 @ [v|1]  (one per head)
            p3a = [a2.tile([P, 2 * W], F32, tag=f"p3a{h}", name=f"p3a{h}",
                           bufs=1) for h in range(HPC)]
            for h in range(HPC):
                nc.vector.memset(p3a[h][:], 0.0)
            CH = 512
            NT = CH // P  # token-tiles per chunk
            GCH = 2048
            zcb = {}
            for c in range(NZ // CH):
                g, sub = divmod(c, GCH // CH)
                if sub == 0:
                    zcb = {k: a2.tile([P, GCH], BF16, tag=f"zc{k}",
                                      name=f"zc{k}", bufs=1) for k in range(KD)}
                    for k in range(KD):
                        eng = nc.sync if k % 2 == 0 else nc.scalar
                        eng.dma_start(
                            out=zcb[k][:],
                            in_=zT[k * P:(k + 1) * P, g * GCH:(g + 1) * GCH])
                zc = [zcb[k][:, sub * CH:(sub + 1) * CH] for k in range(KD)]
                # kT chunk for both packs
                ktc = []
                for p in range(2):
                    kps = a2ps.tile([P, CH], F32, tag="mmps")
                    for k in range(KD):
                        nc.tensor.matmul(
                            kps[:], wk_sb[:, k, p * P:(p + 1) * P],
                            zc[k], start=(k == 0), stop=(k == KD - 1))
                    ksb = a2.tile([P, CH], BF16, tag=f"ktc{p}")
                    nc.scalar.copy(out=ksb[:], in_=kps[:])
                    ktc.append(ksb)
                    # partial k_land: this chunk covers landmarks c*32..c*32+31
                    ksum = a2.tile([P, CH // LZ], F32, tag=f"ksum{p}")
                    nc.vector.tensor_reduce(
                        out=ksum[:],
                        in_=kps[:].rearrange("p (m l) -> p m l", l=LZ),
                        axis=mybir.AxisListType.X, op=ALU.add)
                    nc.vector.tensor_scalar_mul(
                        k_lT[p][:, c * (CH // LZ):(c + 1) * (CH // LZ)],
                        ksum[:], 1.0 / LZ)
                # v chunk: [tok, 4 heads * 64] -> v_sb [128, h, 66] per tile
                vtiles = []
                for t in range(NT):
                    vps = a2ps.tile([P, GD], F32, tag="mmps")
                    for k in range(KD):
                        nc.tensor.matmul(
                            vps[:], zc[k][:, t * P:(t + 1) * P],
                            wv_sb[:, k, :], start=(k == 0),
                            stop=(k == KD - 1))
                    vsb = a2.tile([P, HPC, W], BF16, tag=f"vsb{t}")
                    nc.vector.tensor_copy(
                        vsb[:, :, 0:DH],
                        vps[:].rearrange("p (h d) -> p h d", d=DH))
                    nc.vector.tensor_copy(vsb[:, :, DH:W], onez_b[:])
                    vtiles.append(vsb)
                # sim3T / E3T -> chunk-partial P3 (sequential mb groups per
                # head keep one open psum accumulation group per bank)
                for h in range(HPC):
                    p, off = h // 2, (h % 2) * DH
                    e3s = []
                    for t in range(NT):
                        s3 = a2ps.tile([P, M], F32, tag="s3ps")
                        nc.tensor.matmul(
                            s3[:],
                            ktc[p][off:off + DH, t * P:(t + 1) * P],
                            q_lTb[p][off:off + DH, :],
                            start=True, stop=True)
                        e3 = a2.tile([P, M], BF16, tag="e3", bufs=2 * NT)
                        nc.scalar.activation(e3[:], s3[:], AF.Exp)
                        e3s.append(e3)
                    p3c = p3ps.tile([P, 2 * W], F32, tag="p3c")
                    for mb in range(2):
                        for t in range(NT):
                            nc.tensor.matmul(
                                p3c[:, mb * W:(mb + 1) * W],
                                e3s[t][:, mb * P:(mb + 1) * P],
                                vtiles[t][:, h, :],
                                start=(t == 0), stop=(t == NT - 1))
                    nc.vector.tensor_add(p3a[h][:], p3a[h][:], p3c[:])
            for p in range(2):
                nc.vector.tensor_copy(k_lTb[p][:], k_lT[p][:])
            # P3 -> A3V  (attn3 @ v with softmax denom from the ones column)
            for h in range(HPC):
                p3v = p3a[h][:].rearrange("p (m w) -> p m w", w=W)
                r3 = a2.tile([P, 2], F32, tag="r3")
                for mb in range(2):
                    nc.vector.reciprocal(
                        r3[:, mb:mb + 1], p3v[:, mb, DH:DH + 1])
                    nc.vector.tensor_scalar_mul(
                        a3v_sb[:, h, mb, :], p3v[:, mb, 0:DH], r3[:, mb:mb + 1])

        # ============ phase B: sim2, attn2(T), pinv, W2e per head ============
        with tc.tile_pool(name="pb", bufs=2) as pb, \
             tc.tile_pool(name="bmis", bufs=1, space="PSUM") as bmis, \
             tc.tile_pool(name="bpin", bufs=1, space="PSUM") as bpin:
            for h in range(HPC):
                p, off = h // 2, (h % 2) * DH
                qh = q_lT[p][off:off + DH, :]   # [64, 256]
                kh = k_lT[p][off:off + DH, :]   # [64, 256]

                sim2 = bmis.tile([P, 2 * M], F32, tag="bmm")
                for mb in range(2):
                    nc.tensor.matmul(
                        sim2[:, mb * M:(mb + 1) * M],
                        qh[:, mb * P:(mb + 1) * P], kh,
                        start=True, stop=True)
                e2 = pb.tile([P, 2 * M], F32, tag="e2")
                s2 = pb.tile([P, 2], F32, tag="s2")
                for mb in range(2):
                    nc.scalar.activation(
                        e2[:, mb * M:(mb + 1) * M], sim2[:, mb * M:(mb + 1) * M],
                        AF.Exp, accum_out=s2[:, mb:mb + 1])
                r2 = pb.tile([P, 2], F32, tag="r2")
                nc.vector.reciprocal(r2[:], s2[:])

                sim2t = bmis.tile([P, 2 * M], F32, tag="bmm")
                for mb in range(2):
                    nc.tensor.matmul(
                        sim2t[:, mb * M:(mb + 1) * M],
                        kh[:, mb * P:(mb + 1) * P], qh,
                        start=True, stop=True)
                e2t = pb.tile([P, 2 * M], F32, tag="e2t")
                nc.scalar.activation(e2t[:], sim2t[:], AF.Exp)

                # row-vector of 1/s2 via PE transpose, then partition bcast
                trp = bmis.tile([1, M], F32, tag="trp")
                for mb in range(2):
                    nc.tensor.transpose(
                        trp[0:1, mb * P:(mb + 1) * P], r2[:, mb:mb + 1],
                        id128[:])
                r2row = pb.tile([1, M], F32, tag="r2row")
                nc.vector.tensor_copy(r2row[:], trp[:])
                r2b = pb.tile([P, M], F32, tag="r2b")
                nc.gpsimd.partition_broadcast(r2b[:], r2row[:])

                # aT = attn2^T ; zt0 = aT*c0 ; ztT0 = attn2*c0
                for mb in range(2):
                    nc.vector.tensor_mul(
                        aT_sb[h][:, mb * M:(mb + 1) * M],
                        e2t[:, mb * M:(mb + 1) * M], r2b[:])
                    nc.vector.tensor_scalar(
                        ztT_sb[h][:, mb * M:(mb + 1) * M],
                        e2[:, mb * M:(mb + 1) * M],
                        r2[:, mb:mb + 1], c0col[:, 0:1],
                        ALU.mult, ALU.mult)
                nc.vector.tensor_scalar_mul(
                    zt_sb[h][:], aT_sb[h][:], c0col[:, 0:1])

            def mm256(out_ps, lhsT_pk, rhs_pk):
                """[256,256] @ [256,256] in packed [128,512] layout."""
                for mb in range(2):
                    for k in range(2):
                        nc.tensor.matmul(
                            out_ps[:, mb * M:(mb + 1) * M],
                            lhsT_pk[:, k * M + mb * P:k * M + (mb + 1) * P],
                            rhs_pk[:, k * M:(k + 1) * M],
                            start=(k == 0), stop=(k == 1))

            ITERS = 6
            for it in range(ITERS):
                for h in range(HPC):
                    az = bpin.tile([P, 2 * M], F32, tag="az")
                    mm256(az, aT_sb[h][:], zt_sb[h][:])
                    azt_ps = bpin.tile([P, 2 * M], F32, tag="azt")
                    mm256(azt_ps, zt_sb[h][:], aT_sb[h][:])
                    azt = pb.tile([P, 2 * M], BF16, tag="azt_sb")
                    nc.scalar.copy(out=azt[:], in_=azt_ps[:])
                    t1 = pb.tile([P, 2 * M], BF16, tag="t1")
                    nc.vector.tensor_sub(t1[:], ids["i7"][:], az[:])
                    t2 = bpin.tile([P, 2 * M], F32, tag="t2")
                    mm256(t2, azt[:], t1[:])
                    t3 = pb.tile([P, 2 * M], BF16, tag="t3")
                    nc.vector.tensor_sub(t3[:], ids["i15"][:], t2[:])
                    t4 = bpin.tile([P, 2 * M], F32, tag="t4")
                    mm256(t4, azt[:], t3[:])
                    t5 = pb.tile([P, 2 * M], BF16, tag="t5")
                    nc.vector.scalar_tensor_tensor(
                        t5[:], t4[:], -0.25, ids["i325"][:],
                        ALU.mult, ALU.add)
                    znew = bpin.tile([P, 2 * M], F32, tag="znew")
                    mm256(znew, ztT_sb[h][:], t5[:])
                    nc.scalar.copy(out=zt_sb[h][:], in_=znew[:])
                    # ztT via PE transpose of the 4 [128,128] blocks
                    ztt_ps = bpin.tile([P, 2 * M], BF16, tag="zttp")
                    for mb in range(2):
                        for k in range(2):
                            nc.tensor.transpose(
                                ztt_ps[:, mb * M + k * P:mb * M + (k + 1) * P],
                                zt_sb[h][:, k * M + mb * P:k * M + (mb + 1) * P],
                                id128b[:])
                    nc.scalar.copy(out=ztT_sb[h][:], in_=ztt_ps[:])

            # W2' = zt @ A3V ; W2e = [W2' | ones] as lhsT [256 -> k, 65]
            for h in range(HPC):
                w2p = bmis.tile([P, 2 * DH], F32, tag="bmm")
                for mb in range(2):
                    for k in range(2):
                        nc.tensor.matmul(
                            w2p[:, mb * DH:(mb + 1) * DH],
                            ztT_sb[h][:, k * M + mb * P:k * M + (mb + 1) * P],
                            a3v_sb[:, h, k, :],
                            start=(k == 0), stop=(k == 1))
                nc.vector.tensor_copy(
                    w2e_sb[:, h, :, 0:DH],
                    w2p[:].rearrange("p (m d) -> p m d", d=DH))
                nc.vector.tensor_copy(w2e_sb[:, h, :, DH:DH + 2], onez_b[:, 0:2, :])

        # ========= phase C: attn1(T) + out projection + residual =========
        with tc.tile_pool(name="pc", bufs=2) as pc, \
             tc.tile_pool(name="cps", bufs=2, space="PSUM") as cps, \
             tc.tile_pool(name="s1ps", bufs=4, space="PSUM") as s1ps:
            CH = 512
            for c in range(N // CH):
                ot = [pc.tile([P, CH], BF16, tag=f"ot{k}", name=f"ot{k}") for k in range(2)]
                for h in range(HPC):
                    p, off = h // 2, (h % 2) * DH
                    e1 = []
                    for mb in range(2):
                        s1 = s1ps.tile([P, CH], F32, tag="s1")
                        nc.tensor.matmul(
                            s1[:],
                            k_lTb[p][off:off + DH, mb * P:(mb + 1) * P],
                            qt_sb[p][off:off + DH, c * CH:(c + 1) * CH],
                            start=True, stop=True)
                        e = pc.tile([P, CH], BF16, tag="e1", bufs=8)
                        nc.scalar.activation(e[:], s1[:], AF.Exp)
                        e1.append(e)
                    otp = cps.tile([P, CH], F32, tag="otp")
                    for k in range(2):
                        nc.tensor.matmul(
                            otp[0:DH + 2, :], w2e_sb[:, h, k, :],
                            e1[k][:], start=(k == 0), stop=(k == 1))
                    r1 = pc.tile([1, CH], F32, tag="r1", bufs=4)
                    nc.vector.reciprocal(r1[:], otp[DH:DH + 1, :])
                    s1b = pc.tile([DH, CH], F32, tag="s1b", bufs=4)
                    nc.gpsimd.partition_broadcast(s1b[:], r1[:])
                    nc.vector.tensor_mul(
                        ot[p][off:off + DH, :], otp[0:DH, :], s1b[:])
                # projection: y[tok, :] = outT.T @ wo + b_out (residual on host)
                ysb = pc.tile([P, CH // P, DIM], BF16, tag="ysb")
                for s in range(CH // P):
                    yps = cps.tile([P, DIM], F32, tag="yps")
                    for k in range(2):
                        nc.tensor.matmul(
                            yps[:], ot[k][:, s * P:(s + 1) * P],
                            wo_sb[:, k, :], start=(k == 0), stop=(k == 1))
                    nc.vector.tensor_add(ysb[:, s, :], yps[:], bbcast[:])
                nc.sync.dma_start(
                    out=y[c * CH:(c + 1) * CH, :].rearrange(
                        "(s p) d -> p s d", p=P),
                    in_=ysb[:])

        persist.release()
        const.release()
    return nc


_BUILT = None


def _build():
    global _BUILT
    if _BUILT is None:
        nc = bacc.Bacc("TRN2", target_bir_lowering=False, debug=False)
        _emit(nc)
        nc.finalize()
        _BUILT = nc
    return _BUILT


def _host_c0(x, z, W_q, W_kv):
    """Replicate the reference's global pinv z0 scale (one scalar)."""
    x_land = x.reshape(B, M, L, DIM).mean(2)
    z_land = z.reshape(B, M, LZ, DIM).mean(2)
    q_land = (x_land @ W_q) * SCALE          # [B, M, H*DH]
    k_land = z_land @ W_kv[:, :H * DH]
    q_land = q_land.reshape(B, M, H, DH).transpose(0, 2, 1, 3)
    k_land = k_land.reshape(B, M, H, DH).transpose(0, 2, 1, 3)
    sim2 = q_land @ k_land.transpose(0, 1, 3, 2)   # [B, H, M, M]
    e = np.exp(sim2 - sim2.max(-1, keepdims=True))
    attn2 = e / e.sum(-1, keepdims=True)
    col = np.abs(attn2).sum(-1)
    row = np.abs(attn2).sum(-2)
    return np.float32(1.0) / (col.max() * row.max())


def kernel(x, z, W_q, W_kv, W_out, b_out):
    global LAST_RESULTS, LAST_IN_MAPS
    x = np.asarray(x, dtype=np.float32)
    z = np.asarray(z, dtype=np.float32)
    W_q = np.asarray(W_q, dtype=np.float32)
    W_kv = np.asarray(W_kv, dtype=np.float32)
    W_out = np.asarray(W_out, dtype=np.float32)
    b_out = np.asarray(b_out, dtype=np.float32)

    c0 = np.array([[_host_c0(x, z, W_q, W_kv)]], dtype=np.float32)
    zeros_row = np.zeros((1, DIM), dtype=np.float32)
    bb_row = b_out.reshape(1, DIM).astype(np.float32)

    bf = ml_dtypes.bfloat16
    in_maps = []
    for i in range(N_CORES):
        bi, g = i // 2, i % 2
        cs = slice(g * GD, (g + 1) * GD)
        w3cat = np.concatenate(
            [W_q[:, cs] * SCALE, W_kv[:, cs], W_kv[:, H * DH:][:, cs]], axis=1)
        in_maps.append({
            "xT": np.ascontiguousarray(x[bi].T).astype(bf),
            "zT": np.ascontiguousarray(z[bi].T).astype(bf),
            "w3": np.ascontiguousarray(w3cat).astype(bf),
            "wo": np.ascontiguousarray(W_out[cs, :]).astype(bf),
            "c0": c0,
            "bb": zeros_row if g == 0 else bb_row,
        })

    nc = _build()
    LAST_IN_MAPS = in_maps
    LAST_RESULTS = run_bass_kernel_spmd(
        nc, in_maps, list(range(N_CORES)),
        tmpdir=os.environ.get("KERNEL_TRACE_DIR") or None)
    rs = LAST_RESULTS.results

    out = np.empty((B, N, DIM), dtype=np.float32)
    for bi in range(B):
        np.add(rs[2 * bi]["y"].astype(np.float32),
               rs[2 * bi + 1]["y"].astype(np.float32), out=out[bi])
        out[bi] += x[bi]
    return out
